# revision 4
# baseline (speedup 1.0000x reference)
"""AdaptivePatchSequenceBuilder Trainium2 kernel (8 NeuronCores, SPMD data-parallel).

Strategy
--------
Shard the 2048x2048 image by horizontal stripes: core c owns image rows
[256c, 256c+256) == 16-px-patch grid rows [16c,16c+16) == 32-px grid rows
[8c, 8c+8).  Each core computes dense patch embeddings for all its grid
positions (both scales), adds the (bias-folded) positional embeddings, and
writes dense [token, 768] outputs.  The host then compacts rows by the
(static) masks and concatenates - pure gather/unshard glue.

Device pipeline per core:
  1. SWDGE DMA with f32->bf16 cast: image stripe -> SBUF rows-layout S1.
  2. Round-1 PE transposes: S1 [(rows),(gx,w)] -> U [gx, (gy',c,h,w)]
     (one 128x128 transpose per (c, row-group, w)).
     For scale-2: same step but with a vertical-pooling matrix as the
     matmul RHS and two horizontally-shifted stationary slices accumulated
     in PSUM => fused 2x2 avg-pool + transpose (bilinear/antialias=False
     downscale is exactly 2x2 avg-pooling).
  3. Round-2 PE transposes: U -> X^T tiles [k=(c,h,w) chunk, patch].
  4. GEMM: out[patch,d] += X^T_chunk.T @ W_chunk (bf16, f32 PSUM accum).
  5. DVE: psum + pos' -> out tile (pos' = pos embed with conv bias folded
     in on the host; for scale-2, the bicubically resized pos embed).
  6. HWDGE DMA out.

zc_w / zc_b are zero-filled per the module spec, so the patch_attn branch
contributes exactly zero and is skipped (a host-side numpy fallback handles
the hypothetical nonzero case).
"""

import numpy as np

IMG = 2048
P = 16
D = 768
G = 128          # 16px-patch grid
G2 = 64          # 32px-patch grid
NCORES = 8
RPC = G // NCORES      # 16 grid rows per core
R2PC = G2 // NCORES    # 8 grid2 rows per core

_CACHE = {}


# --------------------------------------------------------------------------
# host-side math helpers
# --------------------------------------------------------------------------

def _keys_cubic(x):
    x = np.abs(x)
    a = -0.5
    return np.where(
        x <= 1, (a + 2) * x**3 - (a + 3) * x**2 + 1,
        np.where(x < 2, a * (x**3 - 5 * x**2 + 8 * x - 4), 0.0))


def _resize_mat(insz, outsz):
    # replicates jax.image.resize(..., 'bicubic', antialias=True) weights
    scale = outsz / insz
    sample_f = (np.arange(outsz) + 0.5) / scale - 0.5
    x = np.abs(sample_f[:, None] - np.arange(insz)[None, :]) * scale
    w = _keys_cubic(x)
    w = w / w.sum(axis=1, keepdims=True)
    return w.astype(np.float32)


def _bicubic_resize_pos(pg):
    # pg: (G, G, D) f32 -> (G2, G2, D) f32, matching jax bicubic
    A = _resize_mat(G, G2)
    t = (A @ pg.reshape(G, G * D)).reshape(G2, G, D)           # rows
    t = np.einsum('bj,ajd->abd', A, t, optimize=True)          # cols
    return t.astype(np.float32)


def _attn_branch_numpy(image, mask32, proj_w, proj_b, attn_w, attn_b, zc_w, zc_b):
    """Fallback for nonzero zero-conv params (never hit for the spec'd module)."""
    m32 = np.asarray(mask32)
    img = np.asarray(image)[0]
    base = img.reshape(3, G, P, G, P).transpose(1, 3, 0, 2, 4)      # (G,G,3,P,P)
    grouped = base.reshape(G2, 2, G2, 2, 3, P, P).transpose(0, 2, 1, 3, 4, 5, 6)
    grouped = grouped.reshape(G2, G2, 4, 3, P, P)[m32]              # (N2,4,3,P,P)
    wf = proj_w.reshape(D, -1)
    ft = grouped.reshape(-1, 3 * P * P) @ wf.T + proj_b
    ft = ft.reshape(-1, 2, 2, D).transpose(0, 3, 1, 2)
    attn = ft.reshape(ft.shape[0], -1) @ attn_w.reshape(D, -1).T + attn_b
    return attn @ zc_w.T + zc_b


# --------------------------------------------------------------------------
# device program
# --------------------------------------------------------------------------

def _build_program():
    import concourse.bass as bass
    import concourse.bacc as bacc
    import concourse.tile as tile
    mybir = bass.mybir
    f32 = mybir.dt.float32
    bf16 = mybir.dt.bfloat16

    nc = bacc.Bacc("TRN2", target_bir_lowering=False, debug=False,
                   num_devices=NCORES)

    img = nc.dram_tensor("img", [3, 256, IMG], f32, kind="ExternalInput").ap()
    poss = nc.dram_tensor("poss", [RPC, G, D], f32, kind="ExternalInput").ap()
    pos32r = nc.dram_tensor("pos32r", [R2PC // 2, G, D], f32,
                            kind="ExternalInput").ap()
    wq = nc.dram_tensor("wq", [6, 128, D], bf16, kind="ExternalInput").ap()
    mpool = nc.dram_tensor("mpool", [128, 64], bf16, kind="ExternalInput").ap()
    ident = nc.dram_tensor("ident", [128, 128], bf16, kind="ExternalInput").ap()
    o16 = nc.dram_tensor("o16", [RPC, G, D], f32, kind="ExternalOutput").ap()
    o32 = nc.dram_tensor("o32", [R2PC // 2, G, D], f32, kind="ExternalOutput").ap()

    add = mybir.AluOpType.add

    with tile.TileContext(nc) as tc:
        with (
            tc.tile_pool(name="const", bufs=1) as constp,
            tc.tile_pool(name="s1", bufs=2) as s1p,
            tc.tile_pool(name="u", bufs=2) as up,
            tc.tile_pool(name="u32", bufs=2) as u32p,
            tc.tile_pool(name="xt", bufs=4) as xtp,
            tc.tile_pool(name="xt32", bufs=2) as xt32p,
            tc.tile_pool(name="pos", bufs=4) as posp,
            tc.tile_pool(name="out", bufs=4) as outp,
            tc.tile_pool(name="ps1", bufs=2, space="PSUM") as ps1p,
            tc.tile_pool(name="ps1b", bufs=1, space="PSUM") as ps1bp,
            tc.tile_pool(name="ps2", bufs=1, space="PSUM") as ps2p,
            tc.tile_pool(name="psg", bufs=1, space="PSUM") as psgp,
        ):
            ident_t = constp.tile([128, 128], bf16)
            nc.sync.dma_start(ident_t[:], ident[:])
            mpool_t = constp.tile([128, 64], bf16)
            nc.sync.dma_start(mpool_t[:], mpool[:])
            wq_t = []
            for j in range(6):
                w_ = constp.tile([128, D], bf16, tag=f"wq{j}")
                nc.sync.dma_start(w_[:], wq[j])
                wq_t.append(w_)

            for grp in range(2):
                # ---- image stripe (rows-major, bf16 cast during DMA) ----
                s1 = []
                for c in range(3):
                    t = s1p.tile([128, IMG], bf16, tag=f"s1c{c}")
                    nc.gpsimd.dma_start(t[:], img[c, 128 * grp:128 * (grp + 1), :])
                    s1.append(t)

                # ---- round 1, full res: S1 -> U -----------------------------
                # U free layout: gy_lo*768 + c*256 + h*16 + w   (gy_lo in [0,8))
                u = up.tile([128, 8 * D], bf16, tag="u")
                uv = u.rearrange("p (gy c h w) -> p c w gy h", gy=8, c=3, h=16, w=16)
                for c in range(3):
                    s1v = s1[c].rearrange("p (gx w) -> p w gx", w=16)
                    for wquad in range(4):
                        ps = ps1p.tile([128, 512], bf16, tag="ps1")
                        for wsub in range(4):
                            w = 4 * wquad + wsub
                            nc.tensor.transpose(
                                ps[:, 128 * wsub:128 * (wsub + 1)],
                                s1v[:, w, :], ident_t[:])
                        src = ps.rearrange("p (w gy h) -> p w gy h", w=4, gy=8)
                        nc.vector.tensor_copy(
                            uv[:, c, 4 * wquad:4 * (wquad + 1), :, :], src[:])

                # ---- round 1, pooled scale: S1 -> U32 (fused 2x2 avgpool) ---
                # U32 free layout: q*768 + c*256 + (8*par+hp)*16 + w'
                #   where local gy2 = 4*grp+q, pooled row h' = 8*par + hp
                u32 = u32p.tile([64, 4 * D], bf16, tag="u32")
                u32v = u32.rearrange("p (q c par hp w) -> p c w q par hp",
                                     q=4, c=3, par=2, hp=8, w=16)
                for c in range(3):
                    s1w = s1[c].rearrange("p (gx2 t) -> p t gx2", t=32)
                    for w8 in range(2):
                        ps = ps1bp.tile([64, 512], f32, tag="ps1b")
                        for wsub in range(8):
                            wp = 8 * w8 + wsub
                            for delta in range(2):
                                nc.tensor.matmul(
                                    ps[:, 64 * wsub:64 * (wsub + 1)],
                                    s1w[:, 2 * wp + delta, :], mpool_t[:],
                                    start=(delta == 0), stop=(delta == 1))
                        src = ps.rearrange("p (w q par hp) -> p w q par hp",
                                           w=8, q=4, par=2)
                        nc.vector.tensor_copy(
                            u32v[:, c, 8 * w8:8 * (w8 + 1), :, :, :], src[:])

                # ---- round 2 + GEMM + pos-add + out, full res ---------------
                for gy_lo in range(8):
                    t16 = 8 * grp + gy_lo
                    xt = xtp.tile([128, 6 * 128], bf16, tag="xt")
                    psa = ps2p.tile([128, 512], bf16, tag="ps2a")
                    psb = ps2p.tile([128, 256], bf16, tag="ps2b")
                    for j in range(6):
                        dst = psa[:, 128 * j:128 * (j + 1)] if j < 4 else \
                            psb[:, 128 * (j - 4):128 * (j - 3)]
                        nc.tensor.transpose(
                            dst, u[:, D * gy_lo + 128 * j:D * gy_lo + 128 * (j + 1)],
                            ident_t[:])
                    nc.vector.tensor_copy(xt[:, 0:512], psa[:])
                    nc.vector.tensor_copy(xt[:, 512:768], psb[:])

                    pga = psgp.tile([128, 512], f32, tag="psga")
                    pgb = psgp.tile([128, 256], f32, tag="psgb")
                    for j in range(6):
                        lhsT = xt[:, 128 * j:128 * (j + 1)]
                        nc.tensor.matmul(pga[:], lhsT, wq_t[j][:, 0:512],
                                         start=(j == 0), stop=(j == 5))
                        nc.tensor.matmul(pgb[:], lhsT, wq_t[j][:, 512:768],
                                         start=(j == 0), stop=(j == 5))
                    pt = posp.tile([128, D], f32, tag="pos")
                    nc.sync.dma_start(pt[:], poss[t16])
                    ot = outp.tile([128, D], f32, tag="out")
                    nc.vector.tensor_tensor(ot[:, 0:512], pga[:], pt[:, 0:512], add)
                    nc.vector.tensor_tensor(ot[:, 512:768], pgb[:], pt[:, 512:768], add)
                    nc.sync.dma_start(o16[t16], ot[:])

                # ---- round 2 + GEMM + pos-add + out, pooled scale -----------
                for pair in range(2):          # two gy2 rows per output tile
                    t32 = 2 * grp + pair
                    xt32 = xt32p.tile([128, 6 * 128], bf16, tag="xt32")
                    for half in range(2):
                        q = 2 * pair + half
                        ps = ps2p.tile([128, 6 * 64], bf16, tag="ps2a")
                        for j in range(6):
                            nc.tensor.transpose(
                                ps[:, 64 * j:64 * (j + 1)],
                                u32[:, D * q + 128 * j:D * q + 128 * (j + 1)],
                                ident_t[:64, :64])
                        src = ps.rearrange("p (j g) -> p j g", j=6)
                        dst = xt32.rearrange("p (j g) -> p j g", j=6)
                        nc.vector.tensor_copy(
                            dst[:, :, 64 * half:64 * (half + 1)], src[:])

                    pga = psgp.tile([128, 512], f32, tag="psga")
                    pgb = psgp.tile([128, 256], f32, tag="psgb")
                    for j in range(6):
                        lhsT = xt32[:, 128 * j:128 * (j + 1)]
                        nc.tensor.matmul(pga[:], lhsT, wq_t[j][:, 0:512],
                                         start=(j == 0), stop=(j == 5))
                        nc.tensor.matmul(pgb[:], lhsT, wq_t[j][:, 512:768],
                                         start=(j == 0), stop=(j == 5))
                    pt = posp.tile([128, D], f32, tag="pos")
                    nc.sync.dma_start(pt[:], pos32r[t32])
                    ot = outp.tile([128, D], f32, tag="out")
                    nc.vector.tensor_tensor(ot[:, 0:512], pga[:], pt[:, 0:512], add)
                    nc.vector.tensor_tensor(ot[:, 512:768], pgb[:], pt[:, 512:768], add)
                    nc.sync.dma_start(o32[t32], ot[:])

    nc.compile()
    return nc


def _get_program():
    if "nc" not in _CACHE:
        _CACHE["nc"] = _build_program()
    return _CACHE["nc"]


# --------------------------------------------------------------------------
# entry point
# --------------------------------------------------------------------------

def prepare_in_maps(image, proj_w, proj_b, base_pos_embed, zc_b):
    import ml_dtypes
    bf16 = ml_dtypes.bfloat16

    # weights in X^T k-order: k = c*256 + h*16 + w
    wq = np.ascontiguousarray(
        proj_w.reshape(D, 3 * P * P).T.reshape(6, 128, D)).astype(bf16)

    pos_seq = base_pos_embed[0, 1:, :]                       # (G*G, D)
    pg2 = _bicubic_resize_pos(pos_seq.reshape(G, G, D))      # (G2, G2, D)
    pos16p = pos_seq.reshape(G, G, D) + proj_b               # bias folded
    pos32p = pg2 + proj_b + zc_b

    # vertical pooling matrix: rows (gy_lo,h) -> pooled (gy_lo, h//2), *0.25
    mp = np.zeros((128, 64), np.float32)
    rho = np.arange(128)
    mp[rho, (rho // 16) * 8 + (rho % 16) // 2] = 0.25
    mp = mp.astype(bf16)
    ident = np.eye(128, dtype=bf16)

    in_maps = []
    for c in range(NCORES):
        in_maps.append({
            "img": np.ascontiguousarray(image[0, :, 256 * c:256 * (c + 1), :]),
            "poss": np.ascontiguousarray(pos16p[RPC * c:RPC * (c + 1)]
                                         ).reshape(RPC, G, D),
            "pos32r": np.ascontiguousarray(pos32p[R2PC * c:R2PC * (c + 1)]
                                           ).reshape(R2PC // 2, G, D),
            "wq": wq,
            "mpool": mp,
            "ident": ident,
        })
    return in_maps


def kernel(image, mask16, mask32, proj_w, proj_b, cls_token, base_pos_embed,
           attn_w, attn_b, zc_w, zc_b):
    from concourse.bass_utils import run_bass_kernel_spmd

    image = np.asarray(image, dtype=np.float32)
    m16 = np.asarray(mask16).astype(bool)
    m32 = np.asarray(mask32).astype(bool)
    proj_w = np.asarray(proj_w, dtype=np.float32)
    proj_b = np.asarray(proj_b, dtype=np.float32)
    cls_token = np.asarray(cls_token, dtype=np.float32)
    base_pos_embed = np.asarray(base_pos_embed, dtype=np.float32)
    zc_w = np.asarray(zc_w, dtype=np.float32)
    zc_b = np.asarray(zc_b, dtype=np.float32)

    nc = _get_program()
    in_maps = prepare_in_maps(image, proj_w, proj_b, base_pos_embed, zc_b)

    res = run_bass_kernel_spmd(nc, in_maps, core_ids=list(range(NCORES)))
    _CACHE["last_result"] = res
    outs = res.results

    grid16 = np.concatenate([outs[c]["o16"].reshape(RPC * G, D)
                             for c in range(NCORES)], axis=0)
    grid32 = np.concatenate([outs[c]["o32"].reshape(R2PC * G2, D)
                             for c in range(NCORES)], axis=0)

    tok16 = grid16[m16.ravel()]
    tok32 = grid32[m32.ravel()]

    if np.any(zc_w) or np.any(zc_b):
        # zero-conv params are zero-filled for this module; numpy fallback
        # keeps the kernel faithful if they ever are not.
        tok32 = tok32 - zc_b + _attn_branch_numpy(
            image, m32, proj_w, proj_b,
            np.asarray(attn_w, np.float32), np.asarray(attn_b, np.float32),
            zc_w, zc_b)

    cls_row = (cls_token[0, 0] + base_pos_embed[0, 0]).astype(np.float32)
    seq = np.concatenate([cls_row[None], tok16, tok32], axis=0)[None]
    count = int(tok16.shape[0] + tok32.shape[0])
    return seq, count


# revision 5
# speedup vs baseline: 10.9341x; 10.9341x over previous
"""AdaptivePatchSequenceBuilder Trainium2 kernel (8 NeuronCores, SPMD data-parallel).

Strategy
--------
Shard the 2048x2048 image by horizontal stripes: core c owns image rows
[256c, 256c+256) == 16-px-patch grid rows [16c,16c+16) == 32-px grid rows
[8c, 8c+8).  Each core computes dense patch embeddings for all its grid
positions (both scales), adds the (bias-folded) positional embeddings, and
writes dense [token, 768] outputs.  The host then compacts rows by the
(static) masks and concatenates - pure gather/unshard glue.

Device pipeline per core:
  1. SWDGE DMA with f32->bf16 cast: image stripe -> SBUF rows-layout S1.
  2. Round-1 PE transposes: S1 [(rows),(gx,w)] -> U [gx, (gy',c,h,w)]
     (one 128x128 transpose per (c, row-group, w)).
     For scale-2: same step but with a vertical-pooling matrix as the
     matmul RHS and two horizontally-shifted stationary slices accumulated
     in PSUM => fused 2x2 avg-pool + transpose (bilinear/antialias=False
     downscale is exactly 2x2 avg-pooling).
  3. Round-2 PE transposes: U -> X^T tiles [k=(c,h,w) chunk, patch].
  4. GEMM: out[patch,d] += X^T_chunk.T @ W_chunk (bf16, f32 PSUM accum).
  5. DVE: psum + pos' -> out tile (pos' = pos embed with conv bias folded
     in on the host; for scale-2, the bicubically resized pos embed).
  6. HWDGE DMA out.

zc_w / zc_b are zero-filled per the module spec, so the patch_attn branch
contributes exactly zero and is skipped (a host-side numpy fallback handles
the hypothetical nonzero case).
"""

import numpy as np

IMG = 2048
P = 16
D = 768
G = 128          # 16px-patch grid
G2 = 64          # 32px-patch grid
NCORES = 8
RPC = G // NCORES      # 16 grid rows per core
R2PC = G2 // NCORES    # 8 grid2 rows per core

_CACHE = {}


# --------------------------------------------------------------------------
# host-side math helpers
# --------------------------------------------------------------------------

def _keys_cubic(x):
    x = np.abs(x)
    a = -0.5
    return np.where(
        x <= 1, (a + 2) * x**3 - (a + 3) * x**2 + 1,
        np.where(x < 2, a * (x**3 - 5 * x**2 + 8 * x - 4), 0.0))


def _resize_mat(insz, outsz):
    # replicates jax.image.resize(..., 'bicubic', antialias=True) weights
    scale = outsz / insz
    sample_f = (np.arange(outsz) + 0.5) / scale - 0.5
    x = np.abs(sample_f[:, None] - np.arange(insz)[None, :]) * scale
    w = _keys_cubic(x)
    w = w / w.sum(axis=1, keepdims=True)
    return w.astype(np.float32)


def _bicubic_resize_pos(pg):
    # pg: (G, G, D) f32 -> (G2, G2, D) f32, matching jax bicubic
    A = _resize_mat(G, G2)
    t = (A @ pg.reshape(G, G * D)).reshape(G2, G, D)           # rows
    t = np.einsum('bj,ajd->abd', A, t, optimize=True)          # cols
    return t.astype(np.float32)


def _attn_branch_numpy(image, mask32, proj_w, proj_b, attn_w, attn_b, zc_w, zc_b):
    """Fallback for nonzero zero-conv params (never hit for the spec'd module)."""
    m32 = np.asarray(mask32)
    img = np.asarray(image)[0]
    base = img.reshape(3, G, P, G, P).transpose(1, 3, 0, 2, 4)      # (G,G,3,P,P)
    grouped = base.reshape(G2, 2, G2, 2, 3, P, P).transpose(0, 2, 1, 3, 4, 5, 6)
    grouped = grouped.reshape(G2, G2, 4, 3, P, P)[m32]              # (N2,4,3,P,P)
    wf = proj_w.reshape(D, -1)
    ft = grouped.reshape(-1, 3 * P * P) @ wf.T + proj_b
    ft = ft.reshape(-1, 2, 2, D).transpose(0, 3, 1, 2)
    attn = ft.reshape(ft.shape[0], -1) @ attn_w.reshape(D, -1).T + attn_b
    return attn @ zc_w.T + zc_b


# --------------------------------------------------------------------------
# device program
# --------------------------------------------------------------------------

def _build_program(reps=1):
    import concourse.bass as bass
    import concourse.bacc as bacc
    import concourse.tile as tile
    mybir = bass.mybir
    f32 = mybir.dt.float32
    bf16 = mybir.dt.bfloat16

    nc = bacc.Bacc("TRN2", target_bir_lowering=False, debug=False,
                   num_devices=NCORES)

    img = nc.dram_tensor("img", [3, 256, IMG], f32, kind="ExternalInput").ap()
    poss = nc.dram_tensor("poss", [RPC, G, D], f32, kind="ExternalInput").ap()
    pos32r = nc.dram_tensor("pos32r", [R2PC // 2, G, D], f32,
                            kind="ExternalInput").ap()
    wq = nc.dram_tensor("wq", [6, 128, D], bf16, kind="ExternalInput").ap()
    mpool = nc.dram_tensor("mpool", [128, 64], bf16, kind="ExternalInput").ap()
    ident = nc.dram_tensor("ident", [128, 128], bf16, kind="ExternalInput").ap()
    o16 = nc.dram_tensor("o16", [RPC, G, D], f32, kind="ExternalOutput").ap()
    o32 = nc.dram_tensor("o32", [R2PC // 2, G, D], f32, kind="ExternalOutput").ap()

    add = mybir.AluOpType.add

    with tile.TileContext(nc) as tc:
        with (
            tc.tile_pool(name="const", bufs=1) as constp,
            tc.tile_pool(name="s1", bufs=2) as s1p,
            tc.tile_pool(name="u", bufs=2) as up,
            tc.tile_pool(name="u32", bufs=2) as u32p,
            tc.tile_pool(name="xt", bufs=4) as xtp,
            tc.tile_pool(name="xt32", bufs=2) as xt32p,
            tc.tile_pool(name="pos", bufs=4) as posp,
            tc.tile_pool(name="out", bufs=4) as outp,
            tc.tile_pool(name="ps1", bufs=2, space="PSUM") as ps1p,
            tc.tile_pool(name="ps1b", bufs=1, space="PSUM") as ps1bp,
            tc.tile_pool(name="ps2", bufs=1, space="PSUM") as ps2p,
            tc.tile_pool(name="psg", bufs=1, space="PSUM") as psgp,
        ):
            ident_t = constp.tile([128, 128], bf16)
            nc.sync.dma_start(ident_t[:], ident[:])
            mpool_t = constp.tile([128, 64], bf16)
            nc.sync.dma_start(mpool_t[:], mpool[:])
            wq_t = []
            for j in range(6):
                w_ = constp.tile([128, D], bf16, tag=f"wq{j}")
                nc.sync.dma_start(w_[:], wq[j])
                wq_t.append(w_)

            for grp in [g for _ in range(reps) for g in range(2)]:
                # ---- image stripe (rows-major, bf16 cast during DMA) ----
                s1 = []
                for c in range(3):
                    t = s1p.tile([128, IMG], bf16, tag=f"s1c{c}")
                    nc.gpsimd.dma_start(t[:], img[c, 128 * grp:128 * (grp + 1), :])
                    s1.append(t)

                # ---- round 1, full res: S1 -> U -----------------------------
                # U free layout: gy_lo*768 + c*256 + h*16 + w   (gy_lo in [0,8))
                u = up.tile([128, 8 * D], bf16, tag="u")
                uv = u.rearrange("p (gy c h w) -> p c w gy h", gy=8, c=3, h=16, w=16)
                for c in range(3):
                    s1v = s1[c].rearrange("p (gx w) -> p w gx", w=16)
                    for wquad in range(4):
                        ps = ps1p.tile([128, 512], bf16, tag="ps1")
                        for wsub in range(4):
                            w = 4 * wquad + wsub
                            nc.tensor.transpose(
                                ps[:, 128 * wsub:128 * (wsub + 1)],
                                s1v[:, w, :], ident_t[:])
                        src = ps.rearrange("p (w gy h) -> p w gy h", w=4, gy=8)
                        nc.vector.tensor_copy(
                            uv[:, c, 4 * wquad:4 * (wquad + 1), :, :], src[:])

                # ---- round 1, pooled scale: S1 -> U32 (fused 2x2 avgpool) ---
                # U32 free layout: q*768 + c*256 + (8*par+hp)*16 + w'
                #   where local gy2 = 4*grp+q, pooled row h' = 8*par + hp
                u32 = u32p.tile([64, 4 * D], bf16, tag="u32")
                u32v = u32.rearrange("p (q c par hp w) -> p c w q par hp",
                                     q=4, c=3, par=2, hp=8, w=16)
                for c in range(3):
                    s1w = s1[c].rearrange("p (gx2 t) -> p t gx2", t=32)
                    for w8 in range(2):
                        ps = ps1bp.tile([64, 512], f32, tag="ps1b")
                        for wsub in range(8):
                            wp = 8 * w8 + wsub
                            for delta in range(2):
                                nc.tensor.matmul(
                                    ps[:, 64 * wsub:64 * (wsub + 1)],
                                    s1w[:, 2 * wp + delta, :], mpool_t[:],
                                    start=(delta == 0), stop=(delta == 1))
                        src = ps.rearrange("p (w q par hp) -> p w q par hp",
                                           w=8, q=4, par=2)
                        nc.vector.tensor_copy(
                            u32v[:, c, 8 * w8:8 * (w8 + 1), :, :, :], src[:])

                # ---- round 2 + GEMM + pos-add + out, full res ---------------
                for gy_lo in range(8):
                    t16 = 8 * grp + gy_lo
                    xt = xtp.tile([128, 6 * 128], bf16, tag="xt")
                    psa = ps2p.tile([128, 512], bf16, tag="ps2a")
                    psb = ps2p.tile([128, 256], bf16, tag="ps2b")
                    for j in range(6):
                        dst = psa[:, 128 * j:128 * (j + 1)] if j < 4 else \
                            psb[:, 128 * (j - 4):128 * (j - 3)]
                        nc.tensor.transpose(
                            dst, u[:, D * gy_lo + 128 * j:D * gy_lo + 128 * (j + 1)],
                            ident_t[:])
                    nc.vector.tensor_copy(xt[:, 0:512], psa[:])
                    nc.vector.tensor_copy(xt[:, 512:768], psb[:])

                    pga = psgp.tile([128, 512], f32, tag="psga")
                    pgb = psgp.tile([128, 256], f32, tag="psgb")
                    for j in range(6):
                        lhsT = xt[:, 128 * j:128 * (j + 1)]
                        nc.tensor.matmul(pga[:], lhsT, wq_t[j][:, 0:512],
                                         start=(j == 0), stop=(j == 5))
                        nc.tensor.matmul(pgb[:], lhsT, wq_t[j][:, 512:768],
                                         start=(j == 0), stop=(j == 5))
                    pt = posp.tile([128, D], f32, tag="pos")
                    nc.sync.dma_start(pt[:], poss[t16])
                    ot = outp.tile([128, D], f32, tag="out")
                    nc.vector.tensor_tensor(ot[:, 0:512], pga[:], pt[:, 0:512], add)
                    nc.vector.tensor_tensor(ot[:, 512:768], pgb[:], pt[:, 512:768], add)
                    nc.sync.dma_start(o16[t16], ot[:])

                # ---- round 2 + GEMM + pos-add + out, pooled scale -----------
                for pair in range(2):          # two gy2 rows per output tile
                    t32 = 2 * grp + pair
                    xt32 = xt32p.tile([128, 6 * 128], bf16, tag="xt32")
                    for half in range(2):
                        q = 2 * pair + half
                        ps = ps2p.tile([128, 6 * 64], bf16, tag="ps2a")
                        for j in range(6):
                            nc.tensor.transpose(
                                ps[:, 64 * j:64 * (j + 1)],
                                u32[:, D * q + 128 * j:D * q + 128 * (j + 1)],
                                ident_t[:64, :64])
                        src = ps.rearrange("p (j g) -> p j g", j=6)
                        dst = xt32.rearrange("p (j g) -> p j g", j=6)
                        nc.vector.tensor_copy(
                            dst[:, :, 64 * half:64 * (half + 1)], src[:])

                    pga = psgp.tile([128, 512], f32, tag="psga")
                    pgb = psgp.tile([128, 256], f32, tag="psgb")
                    for j in range(6):
                        lhsT = xt32[:, 128 * j:128 * (j + 1)]
                        nc.tensor.matmul(pga[:], lhsT, wq_t[j][:, 0:512],
                                         start=(j == 0), stop=(j == 5))
                        nc.tensor.matmul(pgb[:], lhsT, wq_t[j][:, 512:768],
                                         start=(j == 0), stop=(j == 5))
                    pt = posp.tile([128, D], f32, tag="pos")
                    nc.sync.dma_start(pt[:], pos32r[t32])
                    ot = outp.tile([128, D], f32, tag="out")
                    nc.vector.tensor_tensor(ot[:, 0:512], pga[:], pt[:, 0:512], add)
                    nc.vector.tensor_tensor(ot[:, 512:768], pgb[:], pt[:, 512:768], add)
                    nc.sync.dma_start(o32[t32], ot[:])

    nc.compile()
    return nc


def _get_program(reps=1):
    key = f"nc{reps}"
    if key not in _CACHE:
        _CACHE[key] = _build_program(reps)
    return _CACHE[key]


# --------------------------------------------------------------------------
# entry point
# --------------------------------------------------------------------------

def prepare_in_maps(image, proj_w, proj_b, base_pos_embed, zc_b):
    import ml_dtypes
    bf16 = ml_dtypes.bfloat16

    # weights in X^T k-order: k = c*256 + h*16 + w
    wq = np.ascontiguousarray(
        proj_w.reshape(D, 3 * P * P).T.reshape(6, 128, D)).astype(bf16)

    pos_seq = base_pos_embed[0, 1:, :]                       # (G*G, D)
    pg2 = _bicubic_resize_pos(pos_seq.reshape(G, G, D))      # (G2, G2, D)
    pos16p = pos_seq.reshape(G, G, D) + proj_b               # bias folded
    pos32p = pg2 + proj_b + zc_b

    # vertical pooling matrix: rows (gy_lo,h) -> pooled (gy_lo, h//2), *0.25
    mp = np.zeros((128, 64), np.float32)
    rho = np.arange(128)
    mp[rho, (rho // 16) * 8 + (rho % 16) // 2] = 0.25
    mp = mp.astype(bf16)
    ident = np.eye(128, dtype=bf16)

    in_maps = []
    for c in range(NCORES):
        in_maps.append({
            "img": np.ascontiguousarray(image[0, :, 256 * c:256 * (c + 1), :]),
            "poss": np.ascontiguousarray(pos16p[RPC * c:RPC * (c + 1)]
                                         ).reshape(RPC, G, D),
            "pos32r": np.ascontiguousarray(pos32p[R2PC * c:R2PC * (c + 1)]
                                           ).reshape(R2PC // 2, G, D),
            "wq": wq,
            "mpool": mp,
            "ident": ident,
        })
    return in_maps


def kernel(image, mask16, mask32, proj_w, proj_b, cls_token, base_pos_embed,
           attn_w, attn_b, zc_w, zc_b):
    from concourse.bass_utils import run_bass_kernel_spmd

    image = np.asarray(image, dtype=np.float32)
    m16 = np.asarray(mask16).astype(bool)
    m32 = np.asarray(mask32).astype(bool)
    proj_w = np.asarray(proj_w, dtype=np.float32)
    proj_b = np.asarray(proj_b, dtype=np.float32)
    cls_token = np.asarray(cls_token, dtype=np.float32)
    base_pos_embed = np.asarray(base_pos_embed, dtype=np.float32)
    zc_w = np.asarray(zc_w, dtype=np.float32)
    zc_b = np.asarray(zc_b, dtype=np.float32)

    nc = _get_program()
    in_maps = prepare_in_maps(image, proj_w, proj_b, base_pos_embed, zc_b)

    res = run_bass_kernel_spmd(nc, in_maps, core_ids=list(range(NCORES)))
    _CACHE["last_result"] = res
    outs = res.results

    grid16 = np.concatenate([outs[c]["o16"].reshape(RPC * G, D)
                             for c in range(NCORES)], axis=0)
    grid32 = np.concatenate([outs[c]["o32"].reshape(R2PC * G2, D)
                             for c in range(NCORES)], axis=0)

    tok16 = grid16[m16.ravel()]
    tok32 = grid32[m32.ravel()]

    if np.any(zc_w) or np.any(zc_b):
        # zero-conv params are zero-filled for this module; numpy fallback
        # keeps the kernel faithful if they ever are not.
        tok32 = tok32 - zc_b + _attn_branch_numpy(
            image, m32, proj_w, proj_b,
            np.asarray(attn_w, np.float32), np.asarray(attn_b, np.float32),
            zc_w, zc_b)

    cls_row = (cls_token[0, 0] + base_pos_embed[0, 0]).astype(np.float32)
    seq = np.concatenate([cls_row[None], tok16, tok32], axis=0)[None]
    count = int(tok16.shape[0] + tok32.shape[0])
    return seq, count


# revision 23
# speedup vs baseline: 413.7770x; 37.8428x over previous
"""AdaptivePatchSequenceBuilder Trainium2 kernel (8 NeuronCores, SPMD data-parallel).

Strategy
--------
Shard the 2048x2048 image by horizontal stripes: core c owns image rows
[256c, 256c+256) == 16-px-patch grid rows [16c,16c+16) == 32-px grid rows
[8c, 8c+8).  Masks are known when kernel() is called, so the program is
traced against them (cached on the mask pattern).

Device pipeline per core:
  1. SWDGE DMA with f32->bf16 cast: image stripe -> SBUF rows-layout S1.
  2. Round-1 PE transposes: S1 [(rows),(gx,w)] -> U [gx, (gy',c,h,w)].
     For the 32px scale the same step uses a vertical-pooling matrix as the
     matmul RHS plus two horizontally-shifted stationary slices accumulated
     in PSUM => fused 2x2 avg-pool + transpose (the bilinear antialias=False
     half-res downscale is exactly 2x2 avg-pooling).
  3. Round-2 "transpose" with a 0/1 column-SELECTION matrix as the RHS:
     emits X^T tiles [k-chunk, patch] holding only mask-SELECTED patches,
     compacted and packed across grid rows into shared 128-patch tiles
     (per-row width = max count across the 8 cores, so the program stays
     SPMD-uniform; narrower cores produce zero columns the host discards).
  4. GEMM per packed tile: out[patch, d] += X^T_chunk.T @ W_chunk
     (bf16 operands, f32 PSUM accumulation).
  5. DVE: psum + packed pos' -> out tile (pos' = positional embedding with
     the conv bias folded in host-side; for the 32px scale the bicubically
     resized pos embedding, packed identically).
  6. HWDGE DMA out (bf16); host upcasts, unpacks, and concatenates.

zc_w / zc_b are zero-filled per the module spec, so the patch_attn branch
contributes exactly zero and is skipped (a host-side numpy fallback handles
the hypothetical nonzero case).
"""

import numpy as np

IMG = 2048
P = 16
D = 768
G = 128          # 16px-patch grid
G2 = 64          # 32px-patch grid
NCORES = 8
RPC = G // NCORES      # 16 grid rows per core
R2PC = G2 // NCORES    # 8 grid2 rows per core

_CACHE = {}


# --------------------------------------------------------------------------
# host-side math helpers
# --------------------------------------------------------------------------

def _keys_cubic(x):
    x = np.abs(x)
    a = -0.5
    return np.where(
        x <= 1, (a + 2) * x**3 - (a + 3) * x**2 + 1,
        np.where(x < 2, a * (x**3 - 5 * x**2 + 8 * x - 4), 0.0))


def _resize_mat(insz, outsz):
    # replicates jax.image.resize(..., 'bicubic', antialias=True) weights
    scale = outsz / insz
    sample_f = (np.arange(outsz) + 0.5) / scale - 0.5
    x = np.abs(sample_f[:, None] - np.arange(insz)[None, :]) * scale
    w = _keys_cubic(x)
    w = w / w.sum(axis=1, keepdims=True)
    return w.astype(np.float32)


def _bicubic_resize_pos(pg):
    # pg: (G, G, D) f32 -> (G2, G2, D) f32, matching jax bicubic
    A = _resize_mat(G, G2)
    t = (A @ pg.reshape(G, G * D)).reshape(G2, G, D)           # rows
    t = np.einsum('bj,ajd->abd', A, t, optimize=True)          # cols
    return t.astype(np.float32)


def _attn_branch_numpy(image, mask32, proj_w, proj_b, attn_w, attn_b, zc_w, zc_b):
    """Fallback for nonzero zero-conv params (never hit for the spec'd module)."""
    m32 = np.asarray(mask32)
    img = np.asarray(image)[0]
    base = img.reshape(3, G, P, G, P).transpose(1, 3, 0, 2, 4)      # (G,G,3,P,P)
    grouped = base.reshape(G2, 2, G2, 2, 3, P, P).transpose(0, 2, 1, 3, 4, 5, 6)
    grouped = grouped.reshape(G2, G2, 4, 3, P, P)[m32]              # (N2,4,3,P,P)
    wf = proj_w.reshape(D, -1)
    ft = grouped.reshape(-1, 3 * P * P) @ wf.T + proj_b
    ft = ft.reshape(-1, 2, 2, D).transpose(0, 3, 1, 2)
    attn = ft.reshape(ft.shape[0], -1) @ attn_w.reshape(D, -1).T + attn_b
    return attn @ zc_w.T + zc_b


# --------------------------------------------------------------------------
# packing plan (mask-derived, SPMD-uniform across cores)
# --------------------------------------------------------------------------

def _pack_plan(nmax, cap=128):
    """Pack per-row slot ranges [0, nmax[r]) into cap-wide tiles.

    Returns (segs, ntiles): segs[r] = [(tile, col_off, slot_lo, slot_hi)].
    """
    segs = []
    tile_i, off = 0, 0
    for n in nmax:
        row = []
        lo = 0
        while n - lo > 0:
            take = min(cap - off, n - lo)
            row.append((tile_i, off, lo, lo + take))
            off += take
            lo += take
            if off == cap:
                tile_i += 1
                off = 0
        segs.append(row)
    return segs, tile_i + (1 if off > 0 else 0)


def _make_plan(m16, m32):
    c16 = m16.reshape(NCORES, RPC, G).sum(axis=2)    # (core, row)
    c32 = m32.reshape(NCORES, R2PC, G2).sum(axis=2)
    # round widths up to even so every PSUM bf16 column offset stays
    # 4-byte aligned (walrus checkMatmultOutputs requirement)
    nmax16 = [(int(c16[:, r].max()) + 1) // 2 * 2 for r in range(RPC)]
    nmax32 = [(int(c32[:, r].max()) + 1) // 2 * 2 for r in range(R2PC)]
    segs16, nt16 = _pack_plan(nmax16)
    segs32, nt32 = _pack_plan(nmax32)
    return dict(nmax16=nmax16, nmax32=nmax32, segs16=segs16, segs32=segs32,
                nt16=nt16, nt32=nt32)


# --------------------------------------------------------------------------
# device program
# --------------------------------------------------------------------------

def _build_program(plan, reps=1):
    import concourse.bass as bass
    import concourse.bacc as bacc
    import concourse.tile as tile
    mybir = bass.mybir
    f32 = mybir.dt.float32
    bf16 = mybir.dt.bfloat16

    nt16, nt32 = plan["nt16"], plan["nt32"]
    segs16, segs32 = plan["segs16"], plan["segs32"]

    nc = bacc.Bacc("TRN2", target_bir_lowering=False, debug=False,
                   num_devices=NCORES)

    img = nc.dram_tensor("img", [3, 256, IMG], f32, kind="ExternalInput").ap()
    poss = nc.dram_tensor("poss", [nt16, 128, D], bf16, kind="ExternalInput").ap()
    pos32r = nc.dram_tensor("pos32r", [nt32, 128, D], bf16,
                            kind="ExternalInput").ap()
    wq = nc.dram_tensor("wq", [6, 128, D], bf16, kind="ExternalInput").ap()
    mpool = nc.dram_tensor("mpool", [128, 64], bf16, kind="ExternalInput").ap()
    ident = nc.dram_tensor("ident", [128, 128], bf16, kind="ExternalInput").ap()
    s16 = nc.dram_tensor("s16", [128, RPC, 128], bf16, kind="ExternalInput").ap()
    s32 = nc.dram_tensor("s32", [64, R2PC, 64], bf16, kind="ExternalInput").ap()
    o16 = nc.dram_tensor("o16", [nt16, 128, D], bf16, kind="ExternalOutput").ap()
    o32 = nc.dram_tensor("o32", [nt32, 128, D], bf16, kind="ExternalOutput").ap()

    add = mybir.AluOpType.add

    # tile -> list of (local_row, col_off, slot_lo, n)
    contrib16 = [[] for _ in range(nt16)]
    for r, row in enumerate(segs16):
        for (t, off, lo, hi) in row:
            contrib16[t].append((r, off, lo, hi - lo))
    contrib32 = [[] for _ in range(nt32)]
    for r, row in enumerate(segs32):
        for (t, off, lo, hi) in row:
            contrib32[t].append((r, off, lo, hi - lo))

    with tile.TileContext(nc) as tc:
        with (
            tc.tile_pool(name="const", bufs=1) as constp,
            tc.tile_pool(name="s1", bufs=2) as s1p,
            tc.tile_pool(name="u", bufs=2) as up,
            tc.tile_pool(name="u32", bufs=2) as u32p,
            tc.tile_pool(name="xt", bufs=6) as xtp,
            tc.tile_pool(name="pos", bufs=6) as posp,
            tc.tile_pool(name="out", bufs=6) as outp,
            tc.tile_pool(name="psT", bufs=4, space="PSUM") as psTp,
            tc.tile_pool(name="psG", bufs=2, space="PSUM") as psGp,
            tc.tile_pool(name="psH", bufs=2, space="PSUM") as psHp,
        ):
            ident_t = constp.tile([128, 128], bf16)
            nc.sync.dma_start(ident_t[:], ident[:])
            mpool_t = constp.tile([128, 64], bf16)
            s16_t = constp.tile([128, RPC * 128], bf16, tag="s16")
            s32_t = constp.tile([64, R2PC * 64], bf16, tag="s32")
            wq_t = []
            for j in range(6):
                wq_j = constp.tile([128, D], bf16, tag=f"wq{j}")
                wq_t.append(wq_j)

            def load_consts():
                nc.sync.dma_start(mpool_t[:], mpool[:])
                nc.sync.dma_start(s16_t[:], s16.rearrange("g r s -> g (r s)"))
                nc.sync.dma_start(s32_t[:], s32.rearrange("g r s -> g (r s)"))
                for j in range(6):
                    nc.sync.dma_start(wq_t[j][:], wq[j])

            # tiles whose contributing rows are all in grp0 can run
            # while grp1's image stripe is still streaming in
            early16 = {t for t in range(nt16)
                       if all(r < 8 for (r, _, _, _) in contrib16[t])}
            early32 = {t for t in range(nt32)
                       if all(r < 4 for (r, _, _, _) in contrib32[t])}

            for rep in range(reps):
                us, u32s = [], []
                for grp in range(2):
                    # ---- image stripe (bf16 cast during DMA) ----------------
                    s1 = []
                    for c in range(3):
                        t = s1p.tile([128, IMG], bf16, tag=f"s1c{c}")
                        if rep == 0 and grp == 0 and c == 0:
                            # split so PE can start on the first half sooner
                            nc.gpsimd.dma_start(t[0:64, :], img[c, 0:64, :])
                            nc.gpsimd.dma_start(t[64:128, :], img[c, 64:128, :])
                        else:
                            nc.gpsimd.dma_start(
                                t[:], img[c, 128 * grp:128 * (grp + 1), :])
                        s1.append(t)
                    if rep == 0 and grp == 0:
                        load_consts()

                    # ---- round 1, full res: S1 -> U -------------------------
                    # U free layout: gy_lo*768 + c*256 + h*16 + w
                    u = up.tile([128, 8 * D], bf16, tag="u")
                    uv = u.rearrange("p (gy c h w) -> p c w gy h",
                                     gy=8, c=3, h=16, w=16)
                    for c in range(3):
                        s1v = s1[c].rearrange("p (gx w) -> p w gx", w=16)
                        split = rep == 0 and grp == 0 and c == 0
                        for w8 in range(2):
                            ps = psTp.tile([128, 1024], bf16, tag="psT")
                            if split:
                                for wsub in range(8):
                                    w = 8 * w8 + wsub
                                    nc.tensor.transpose(
                                        ps[:, 128 * wsub:128 * wsub + 64],
                                        s1v[0:64, w, :], ident_t[0:64, 0:64])
                                for wsub in range(8):
                                    w = 8 * w8 + wsub
                                    nc.tensor.transpose(
                                        ps[:, 128 * wsub + 64:128 * (wsub + 1)],
                                        s1v[64:128, w, :],
                                        ident_t[64:128, 64:128])
                            else:
                                for wsub in range(8):
                                    w = 8 * w8 + wsub
                                    nc.tensor.transpose(
                                        ps[:, 128 * wsub:128 * (wsub + 1)],
                                        s1v[:, w, :], ident_t[:])
                            sv = ps.rearrange("p (w gy h) -> p w gy h", w=8, gy=8)
                            nc.scalar.copy(
                                uv[:, c, 8 * w8:8 * (w8 + 1), :, :], sv[:])
                    us.append(u)

                    # ---- round 1, pooled: S1 -> U32 (fused 2x2 avgpool) -----
                    # U32 free: q*768 + c*256 + (8*par+hp)*16 + w'
                    u32 = u32p.tile([64, 4 * D], bf16, tag="u32")
                    u32v = u32.rearrange("p (q c par hp w) -> p c w q par hp",
                                         q=4, c=3, par=2, hp=8, w=16)
                    for c in range(3):
                        s1w = s1[c].rearrange("p (gx2 t) -> p t gx2", t=32)
                        for w8 in range(2):
                            ps = psGp.tile([64, 512], f32, tag="psG")
                            for wsub in range(8):
                                wp = 8 * w8 + wsub
                                for delta in range(2):
                                    nc.tensor.matmul(
                                        ps[:, 64 * wsub:64 * (wsub + 1)],
                                        s1w[:, 2 * wp + delta, :], mpool_t[:],
                                        start=(delta == 0), stop=(delta == 1))
                            sv = ps.rearrange("p (w q par hp) -> p w q par hp",
                                              w=8, q=4, par=2)
                            nc.vector.tensor_copy(
                                u32v[:, c, 8 * w8:8 * (w8 + 1), :, :, :], sv[:])
                    u32s.append(u32)

                    # ---- packed tiles: select+transpose, GEMM, pos, out -----
                    def emit_tile(tid, contribs, pooled):
                        xt = xtp.tile([128, 6 * 128], bf16, tag="xt")
                        ps = psTp.tile([128, 768], bf16, tag="psT")
                        for j in range(6):
                            for (r, off, lo, n) in contribs:
                                if pooled:
                                    g_, q = r // 4, r % 4
                                    nc.tensor.transpose(
                                        ps[:, 128 * j + off:128 * j + off + n],
                                        u32s[g_][:, D * q + 128 * j:
                                                 D * q + 128 * (j + 1)],
                                        s32_t[:, 64 * r + lo:64 * r + lo + n])
                                else:
                                    g_, gy_ = r // 8, r % 8
                                    nc.tensor.transpose(
                                        ps[:, 128 * j + off:128 * j + off + n],
                                        us[g_][:, D * gy_ + 128 * j:
                                               D * gy_ + 128 * (j + 1)],
                                        s16_t[:, 128 * r + lo:128 * r + lo + n])
                        nc.scalar.copy(xt[:], ps[:])

                        pga = psGp.tile([128, 384], f32, tag="psG")
                        pgb = psHp.tile([128, 384], f32, tag="psH")
                        for j in range(6):
                            lhsT = xt[:, 128 * j:128 * (j + 1)]
                            nc.tensor.matmul(pga[:], lhsT, wq_t[j][:, 0:384],
                                             start=(j == 0), stop=(j == 5))
                            nc.tensor.matmul(pgb[:], lhsT, wq_t[j][:, 384:768],
                                             start=(j == 0), stop=(j == 5))
                        pt = posp.tile([128, D], bf16, tag="pos")
                        psrc = (pos32r if pooled else poss)[tid]
                        nc.sync.dma_start(pt[:], psrc)
                        ot = outp.tile([128, D], bf16, tag="out")
                        nc.vector.tensor_tensor(ot[:, 0:384], pga[:],
                                                pt[:, 0:384], add)
                        nc.vector.tensor_tensor(ot[:, 384:768], pgb[:],
                                                pt[:, 384:768], add)
                        nc.sync.dma_start((o32 if pooled else o16)[tid], ot[:])

                    if grp == 0:
                        # overlap grp0-only tiles with grp1's image ingest
                        for tid in early16:
                            emit_tile(tid, contrib16[tid], pooled=False)
                        for tid in early32:
                            emit_tile(tid, contrib32[tid], pooled=True)

                for tid in range(nt16):
                    if tid not in early16:
                        emit_tile(tid, contrib16[tid], pooled=False)
                for tid in range(nt32):
                    if tid not in early32:
                        emit_tile(tid, contrib32[tid], pooled=True)

    nc.compile()
    return nc


def _get_program(plan, reps=1):
    key = (reps, tuple(plan["nmax16"]), tuple(plan["nmax32"]))
    if key not in _CACHE:
        _CACHE[key] = _build_program(plan, reps)
    return _CACHE[key]


# --------------------------------------------------------------------------
# host-side input prep / output unpack
# --------------------------------------------------------------------------

def prepare_in_maps(plan, image, m16, m32, proj_w, proj_b, base_pos_embed, zc_b):
    import ml_dtypes
    bf16 = ml_dtypes.bfloat16

    wq = np.ascontiguousarray(
        proj_w.reshape(D, 3 * P * P).T.reshape(6, 128, D)).astype(bf16)

    pos_seq = base_pos_embed[0, 1:, :]                       # (G*G, D)
    pg2 = _bicubic_resize_pos(pos_seq.reshape(G, G, D))      # (G2, G2, D)
    pos16p = (pos_seq.reshape(G, G, D) + proj_b).astype(bf16)
    pos32p = (pg2 + proj_b + zc_b).astype(bf16)

    mp = np.zeros((128, 64), np.float32)
    rho = np.arange(128)
    mp[rho, (rho // 16) * 8 + (rho % 16) // 2] = 0.25
    mp = mp.astype(bf16)
    ident = np.eye(128, dtype=bf16)

    nt16, nt32 = plan["nt16"], plan["nt32"]
    segs16, segs32 = plan["segs16"], plan["segs32"]

    in_maps = []
    for c in range(NCORES):
        s16_np = np.zeros((128, RPC, 128), bf16)
        pos_pk = np.zeros((nt16, 128, D), bf16)
        for r in range(RPC):
            sel = np.nonzero(m16[16 * c + r])[0]
            s16_np[sel, r, np.arange(len(sel))] = 1
            for (t, off, lo, hi) in segs16[r]:
                take = sel[lo:min(hi, len(sel))]
                if len(take):
                    pos_pk[t, off:off + len(take)] = pos16p[16 * c + r, take]
        s32_np = np.zeros((64, R2PC, 64), bf16)
        p32_pk = np.zeros((nt32, 128, D), bf16)
        for r in range(R2PC):
            sel = np.nonzero(m32[8 * c + r])[0]
            s32_np[sel, r, np.arange(len(sel))] = 1
            for (t, off, lo, hi) in segs32[r]:
                take = sel[lo:min(hi, len(sel))]
                if len(take):
                    p32_pk[t, off:off + len(take)] = pos32p[8 * c + r, take]

        in_maps.append({
            "img": np.ascontiguousarray(image[0, :, 256 * c:256 * (c + 1), :]),
            "poss": pos_pk,
            "pos32r": p32_pk,
            "wq": wq,
            "mpool": mp,
            "ident": ident,
            "s16": s16_np,
            "s32": s32_np,
        })
    return in_maps


def _unpack(plan, outs, m16, m32):
    """Packed per-core outputs -> (tok16 [N1,D], tok32 [N2,D]) f32."""
    segs16, segs32 = plan["segs16"], plan["segs32"]
    tok16, tok32 = [], []
    for c in range(NCORES):
        a16 = outs[c]["o16"].astype(np.float32)
        for r in range(RPC):
            cnt = int(m16[16 * c + r].sum())
            for (t, off, lo, hi) in segs16[r]:
                take = min(hi, cnt) - lo
                if take > 0:
                    tok16.append(a16[t, off:off + take])
    for c in range(NCORES):
        a32 = outs[c]["o32"].astype(np.float32)
        for r in range(R2PC):
            cnt = int(m32[8 * c + r].sum())
            for (t, off, lo, hi) in segs32[r]:
                take = min(hi, cnt) - lo
                if take > 0:
                    tok32.append(a32[t, off:off + take])
    return np.concatenate(tok16, 0), np.concatenate(tok32, 0)


# --------------------------------------------------------------------------
# entry point
# --------------------------------------------------------------------------

def kernel(image, mask16, mask32, proj_w, proj_b, cls_token, base_pos_embed,
           attn_w, attn_b, zc_w, zc_b):
    from concourse.bass_utils import run_bass_kernel_spmd

    image = np.asarray(image, dtype=np.float32)
    m16 = np.asarray(mask16).astype(bool)
    m32 = np.asarray(mask32).astype(bool)
    proj_w = np.asarray(proj_w, dtype=np.float32)
    proj_b = np.asarray(proj_b, dtype=np.float32)
    cls_token = np.asarray(cls_token, dtype=np.float32)
    base_pos_embed = np.asarray(base_pos_embed, dtype=np.float32)
    zc_w = np.asarray(zc_w, dtype=np.float32)
    zc_b = np.asarray(zc_b, dtype=np.float32)

    plan = _make_plan(m16, m32)
    nc = _get_program(plan)
    in_maps = prepare_in_maps(plan, image, m16, m32, proj_w, proj_b,
                              base_pos_embed, zc_b)

    res = run_bass_kernel_spmd(nc, in_maps, core_ids=list(range(NCORES)))
    _CACHE["last_result"] = res

    tok16, tok32 = _unpack(plan, res.results, m16, m32)

    if np.any(zc_w) or np.any(zc_b):
        # zero-conv params are zero-filled for this module; numpy fallback
        # keeps the kernel faithful if they ever are not.
        tok32 = tok32 - zc_b + _attn_branch_numpy(
            image, m32, proj_w, proj_b,
            np.asarray(attn_w, np.float32), np.asarray(attn_b, np.float32),
            zc_w, zc_b)

    cls_row = (cls_token[0, 0] + base_pos_embed[0, 0]).astype(np.float32)
    seq = np.concatenate([cls_row[None], tok16, tok32], axis=0)[None]
    count = int(tok16.shape[0] + tok32.shape[0])
    return seq, count
